# revision 37
# baseline (speedup 1.0000x reference)
import sys
sys.path.insert(0, "/opt/trn_rl_repo")
import numpy as np
import concourse.bass as bass
from concourse import mybir
from concourse.bass_utils import run_bass_kernel_spmd

F32 = mybir.dt.float32
U8 = mybir.dt.uint8
ADD = mybir.AluOpType.add
MIN = mybir.AluOpType.min

T, B, N = 100, 64, 2048
TT = T + 1
NCORES = 8
B_LOC = B // NCORES
ELEMS = B_LOC * N
P = 128
RPP = ELEMS // P
SEGS = [8, 8, 16, 16, 16, 16, 16, 8, 8, 8, 8]
WARM = 8
ACC_BLOCKS = [(8, 11, 104, 128)]
SP_ORD = [("x", 0), ("r", 1), ("x", 2), ("r", 3), ("x", 4), ("r", 5),
          ("x", 6), ("r", 7), ("x", 8), ("x", 10)]
ACT_ORD = [("r", 0), ("x", 1), ("r", 2), ("x", 3), ("r", 4), ("x", 5),
           ("r", 6), ("x", 7), ("x", 9)]
EMAX = max(SEGS)
NSEG = len(SEGS)
WROWS = sum(SEGS[:WARM])
SENT_HALF = 1.5e38
assert sum(SEGS) == RPP


def _build_nc():
    nc = bass.Bass()
    x_ext = nc.dram_tensor("x", [P * RPP, TT], F32, kind="ExternalInput")
    r_ext = nc.dram_tensor("r", [P * RPP, TT], F32, kind="ExternalInput")
    s_ext = nc.dram_tensor("s", [P * RPP, T], U8, kind="ExternalOutput")

    xvs, rvs, svs = [], [], []
    uoff = []
    off = 0
    for e in SEGS:
        base = off * P
        xvs.append(x_ext[base:base + P * e].rearrange("(p e) t -> p (e t)", p=P, e=e))
        rvs.append(r_ext[base:base + P * e].rearrange("(p e) t -> p (e t)", p=P, e=e))
        svs.append(s_ext[base:base + P * e].rearrange("(p e) t -> p (e t)", p=P, e=e))
        uoff.append(off)
        off += e

    from contextlib import ExitStack
    ctx = ExitStack()
    with (
        nc.sbuf_tensor([P, RPP, TT], F32) as ub,
        nc.sbuf_tensor([P, WROWS, TT], F32) as rb,
        nc.sbuf_tensor([P, 2, EMAX * TT], F32) as zb,
        nc.sbuf_tensor([P, RPP, T], U8) as sb,
        nc.sbuf_tensor([P, 1], F32) as zer1,
        nc.semaphore() as sem_z,
        nc.semaphore() as sem_s,
        nc.semaphore() as sem_o,
        nc.Block(no_gpsimd_drain=True) as block,
    ):
        sem_xs = [ctx.enter_context(nc.semaphore(f"sem_x{i}")) for i in range(NSEG)]
        sem_rs = [ctx.enter_context(nc.semaphore(f"sem_r{i}")) for i in range(WARM)]
        sem_us = [ctx.enter_context(nc.semaphore(f"sem_u{i}"))
                  for i in range(len(ACC_BLOCKS))]
        def issue(eng, kind, i):
            o = uoff[i]
            e = SEGS[i]
            if kind == "x":
                eng.dma_start(ub[:, o:o + e], xvs[i]).then_inc(sem_xs[i], 16)
            else:
                eng.dma_start(rb[:, o:o + e], rvs[i]).then_inc(sem_rs[i], 16)

        @block.sync
        def _(sync):
            for kind, i in SP_ORD:
                issue(sync, kind, i)

        @block.gpsimd
        def _(gpsimd):
            for bi, (slo, shi, rlo, rhi) in enumerate(ACC_BLOCKS):
                for s in range(slo, shi):
                    gpsimd.wait_ge(sem_xs[s], 16)
                nseg = shi - slo
                er = (rhi - rlo) // nseg
                rsrc = r_ext[rlo * P:rhi * P].rearrange(
                    "(s p e) t -> p s (e t)", s=nseg, p=P, e=er)
                gpsimd.dma_start(ub[:, rlo:rhi], rsrc,
                                 accum_op=ADD).then_inc(sem_us[bi], 16)

        @block.vector
        def _(vector):
            nc.vector.memset(zer1[:], 0.0)
            for i in range(NSEG):
                o = uoff[i]
                e = SEGS[i]
                b = i % 2
                if i < WARM:
                    vector.wait_ge(sem_xs[i], 16)
                    vector.wait_ge(sem_rs[i], 16)
                    nc.vector.tensor_tensor(
                        ub[:, o:o + e], ub[:, o:o + e], rb[:, o:o + e], ADD
                    )
                else:
                    for bi, (slo, shi, _, _) in enumerate(ACC_BLOCKS):
                        if slo <= i < shi:
                            vector.wait_ge(sem_us[bi], 16)
                if i >= 2:
                    vector.wait_ge(sem_s, i - 1)
                nc.vector.tensor_tensor_scan(
                    zb[:, b, 0:e * TT],
                    ub[:, o:o + e].rearrange("p a b -> p (a b)"),
                    zer1[:].broadcast_to((P, e * TT)),
                    0.0, ADD, MIN,
                ).then_inc(sem_z, 1)

        @block.scalar
        def _(scalar):
            for kind, i in ACT_ORD:
                issue(scalar, kind, i)
            for i in range(NSEG):
                o = uoff[i]
                e = SEGS[i]
                b = i % 2
                scalar.wait_ge(sem_z, i + 1)
                zv = zb[:, b, 0:e * TT].rearrange("p (e t) -> p e t", e=e, t=TT)
                nc.scalar.activation(
                    sb[:, o:o + e], zv[:, :, 0:T],
                    mybir.ActivationFunctionType.Relu, bias=1.0, scale=1.0e38,
                ).then_inc(sem_s, 1)
                scalar.wait_ge(sem_s, i + 1)
                scalar.dma_start(svs[i], sb[:, o:o + e]).then_inc(sem_o, 16)
            scalar.wait_ge(sem_o, 16 * NSEG)

    ctx.close()
    return nc


_CHAIN = (2.0 ** np.arange(T, dtype=np.float64)).astype(np.float32)


def _seg_pack(a: np.ndarray) -> np.ndarray:
    out = np.empty((P, RPP, TT), dtype=np.float32)
    np.multiply(a, _CHAIN, out=out[:, :, :T])
    out[:, :, T] = SENT_HALF
    blocks = []
    off = 0
    for e in SEGS:
        blocks.append(out[:, off:off + e, :].reshape(P * e, TT))
        off += e
    return np.ascontiguousarray(np.concatenate(blocks, axis=0))


def _seg_unpack(a: np.ndarray) -> np.ndarray:
    out = np.empty((P, RPP, T), dtype=a.dtype)
    off = 0
    row = 0
    for e in SEGS:
        out[:, off:off + e, :] = a[row:row + P * e].reshape(P, e, T)
        off += e
        row += P * e
    return out


def _make_in_maps(inp, rec):
    xt = inp.transpose(1, 2, 0)
    rt = rec.transpose(1, 2, 0)
    maps = []
    for i in range(NCORES):
        xs = _seg_pack(xt[i * B_LOC:(i + 1) * B_LOC].reshape(P, RPP, T))
        rs = _seg_pack(rt[i * B_LOC:(i + 1) * B_LOC].reshape(P, RPP, T))
        maps.append({"x": xs, "r": rs})
    return maps


def _gather(res) -> np.ndarray:
    outs = [
        _seg_unpack(res.results[i]["s"]).reshape(B_LOC, N, T).transpose(2, 0, 1)
        for i in range(NCORES)
    ]
    return np.concatenate(outs, axis=1).astype(np.float32)


def kernel(inp: np.ndarray, rec: np.ndarray) -> np.ndarray:
    inp = np.asarray(inp, dtype=np.float32)
    rec = np.asarray(rec, dtype=np.float32)
    nc = _build_nc()
    res = run_bass_kernel_spmd(nc, _make_in_maps(inp, rec), list(range(NCORES)))
    return _gather(res)


def run_traced(inp, rec, **kw):
    inp = np.asarray(inp, dtype=np.float32)
    rec = np.asarray(rec, dtype=np.float32)
    nc = _build_nc()
    return run_bass_kernel_spmd(nc, _make_in_maps(inp, rec),
                                list(range(NCORES)), trace=True, **kw)


# revision 38
# speedup vs baseline: 1.0313x; 1.0313x over previous
import sys
sys.path.insert(0, "/opt/trn_rl_repo")
import numpy as np
import concourse.bass as bass
from concourse import mybir
from concourse.bass_utils import run_bass_kernel_spmd

F32 = mybir.dt.float32
U8 = mybir.dt.uint8
ADD = mybir.AluOpType.add
MIN = mybir.AluOpType.min

T, B, N = 100, 64, 2048
TT = T + 1
NCORES = 8
B_LOC = B // NCORES
ELEMS = B_LOC * N
P = 128
RPP = ELEMS // P
SEGS = [8, 8, 16, 16, 16, 16, 16, 8, 8, 8, 8]
WARM = 11
ACC_BLOCKS = []
SP_ORD = [("x", 0), ("r", 1), ("x", 2), ("r", 3), ("x", 4), ("r", 5),
          ("x", 6), ("r", 7), ("x", 8), ("r", 9), ("x", 10)]
ACT_ORD = [("r", 0), ("x", 1), ("r", 2), ("x", 3), ("r", 4), ("x", 5),
           ("r", 6), ("x", 7), ("r", 8), ("x", 9), ("r", 10)]
EMAX = max(SEGS)
NSEG = len(SEGS)
WROWS = sum(SEGS[:WARM])
SENT_HALF = 1.5e38
assert sum(SEGS) == RPP


def _build_nc():
    nc = bass.Bass()
    x_ext = nc.dram_tensor("x", [P * RPP, TT], F32, kind="ExternalInput")
    r_ext = nc.dram_tensor("r", [P * RPP, TT], F32, kind="ExternalInput")
    s_ext = nc.dram_tensor("s", [P * RPP, T], U8, kind="ExternalOutput")

    xvs, rvs, svs = [], [], []
    uoff = []
    off = 0
    for e in SEGS:
        base = off * P
        xvs.append(x_ext[base:base + P * e].rearrange("(p e) t -> p (e t)", p=P, e=e))
        rvs.append(r_ext[base:base + P * e].rearrange("(p e) t -> p (e t)", p=P, e=e))
        svs.append(s_ext[base:base + P * e].rearrange("(p e) t -> p (e t)", p=P, e=e))
        uoff.append(off)
        off += e

    from contextlib import ExitStack
    ctx = ExitStack()
    with (
        nc.sbuf_tensor([P, RPP, TT], F32) as ub,
        nc.sbuf_tensor([P, WROWS, TT], F32) as rb,
        nc.sbuf_tensor([P, 2, EMAX * TT], F32) as zb,
        nc.sbuf_tensor([P, RPP, T], U8) as sb,
        nc.sbuf_tensor([P, 1], F32) as zer1,
        nc.semaphore() as sem_z,
        nc.semaphore() as sem_s,
        nc.semaphore() as sem_o,
        nc.Block(no_gpsimd_drain=True) as block,
    ):
        sem_xs = [ctx.enter_context(nc.semaphore(f"sem_x{i}")) for i in range(NSEG)]
        sem_rs = [ctx.enter_context(nc.semaphore(f"sem_r{i}")) for i in range(WARM)]
        sem_us = [ctx.enter_context(nc.semaphore(f"sem_u{i}"))
                  for i in range(len(ACC_BLOCKS))]
        def issue(eng, kind, i):
            o = uoff[i]
            e = SEGS[i]
            if kind == "x":
                eng.dma_start(ub[:, o:o + e], xvs[i]).then_inc(sem_xs[i], 16)
            else:
                eng.dma_start(rb[:, o:o + e], rvs[i]).then_inc(sem_rs[i], 16)

        @block.sync
        def _(sync):
            for kind, i in SP_ORD:
                issue(sync, kind, i)

        @block.gpsimd
        def _(gpsimd):
            for bi, (slo, shi, rlo, rhi) in enumerate(ACC_BLOCKS):
                for s in range(slo, shi):
                    gpsimd.wait_ge(sem_xs[s], 16)
                nseg = shi - slo
                er = (rhi - rlo) // nseg
                rsrc = r_ext[rlo * P:rhi * P].rearrange(
                    "(s p e) t -> p s (e t)", s=nseg, p=P, e=er)
                gpsimd.dma_start(ub[:, rlo:rhi], rsrc,
                                 accum_op=ADD).then_inc(sem_us[bi], 16)

        @block.vector
        def _(vector):
            nc.vector.memset(zer1[:], 0.0)
            for i in range(NSEG):
                o = uoff[i]
                e = SEGS[i]
                b = i % 2
                if i < WARM:
                    vector.wait_ge(sem_xs[i], 16)
                    vector.wait_ge(sem_rs[i], 16)
                    nc.vector.tensor_tensor(
                        ub[:, o:o + e], ub[:, o:o + e], rb[:, o:o + e], ADD
                    )
                else:
                    for bi, (slo, shi, _, _) in enumerate(ACC_BLOCKS):
                        if slo <= i < shi:
                            vector.wait_ge(sem_us[bi], 16)
                if i >= 2:
                    vector.wait_ge(sem_s, i - 1)
                nc.vector.tensor_tensor_scan(
                    zb[:, b, 0:e * TT],
                    ub[:, o:o + e].rearrange("p a b -> p (a b)"),
                    zer1[:].broadcast_to((P, e * TT)),
                    0.0, ADD, MIN,
                ).then_inc(sem_z, 1)

        @block.scalar
        def _(scalar):
            for kind, i in ACT_ORD:
                issue(scalar, kind, i)
            for i in range(NSEG):
                o = uoff[i]
                e = SEGS[i]
                b = i % 2
                scalar.wait_ge(sem_z, i + 1)
                zv = zb[:, b, 0:e * TT].rearrange("p (e t) -> p e t", e=e, t=TT)
                nc.scalar.activation(
                    sb[:, o:o + e], zv[:, :, 0:T],
                    mybir.ActivationFunctionType.Relu, bias=1.0, scale=1.0e38,
                ).then_inc(sem_s, 1)
                scalar.wait_ge(sem_s, i + 1)
                scalar.dma_start(svs[i], sb[:, o:o + e]).then_inc(sem_o, 16)
            scalar.wait_ge(sem_o, 16 * NSEG)

    ctx.close()
    return nc


_CHAIN = (2.0 ** np.arange(T, dtype=np.float64)).astype(np.float32)


def _seg_pack(a: np.ndarray) -> np.ndarray:
    out = np.empty((P, RPP, TT), dtype=np.float32)
    np.multiply(a, _CHAIN, out=out[:, :, :T])
    out[:, :, T] = SENT_HALF
    blocks = []
    off = 0
    for e in SEGS:
        blocks.append(out[:, off:off + e, :].reshape(P * e, TT))
        off += e
    return np.ascontiguousarray(np.concatenate(blocks, axis=0))


def _seg_unpack(a: np.ndarray) -> np.ndarray:
    out = np.empty((P, RPP, T), dtype=a.dtype)
    off = 0
    row = 0
    for e in SEGS:
        out[:, off:off + e, :] = a[row:row + P * e].reshape(P, e, T)
        off += e
        row += P * e
    return out


def _make_in_maps(inp, rec):
    xt = inp.transpose(1, 2, 0)
    rt = rec.transpose(1, 2, 0)
    maps = []
    for i in range(NCORES):
        xs = _seg_pack(xt[i * B_LOC:(i + 1) * B_LOC].reshape(P, RPP, T))
        rs = _seg_pack(rt[i * B_LOC:(i + 1) * B_LOC].reshape(P, RPP, T))
        maps.append({"x": xs, "r": rs})
    return maps


def _gather(res) -> np.ndarray:
    outs = [
        _seg_unpack(res.results[i]["s"]).reshape(B_LOC, N, T).transpose(2, 0, 1)
        for i in range(NCORES)
    ]
    return np.concatenate(outs, axis=1).astype(np.float32)


def kernel(inp: np.ndarray, rec: np.ndarray) -> np.ndarray:
    inp = np.asarray(inp, dtype=np.float32)
    rec = np.asarray(rec, dtype=np.float32)
    nc = _build_nc()
    res = run_bass_kernel_spmd(nc, _make_in_maps(inp, rec), list(range(NCORES)))
    return _gather(res)


def run_traced(inp, rec, **kw):
    inp = np.asarray(inp, dtype=np.float32)
    rec = np.asarray(rec, dtype=np.float32)
    nc = _build_nc()
    return run_bass_kernel_spmd(nc, _make_in_maps(inp, rec),
                                list(range(NCORES)), trace=True, **kw)
